# revision 4
# baseline (speedup 1.0000x reference)
"""Trainium2 Bass kernel for nn_Charge_Fusion (cross-attention charge fusion).

Math (reference, per fact q and label c):
    q    = Q_fact @ W_fact.T + b_fact                       [Q, H]
    cemb = charge @ W_charge.T + b_charge                   [C, S, H]
    attn = softmax_s(q . cemb + mask)                       [Q, C, S]
    emb  = attn @ cemb                                      [Q, C, H]
    out  = sum_h(tanh((q + emb) @ W_fusion.T + b_fusion) * Ws + bias)   [Q, C]

Algebraic rewrite used here (saves the dominant 121-GF charge projection):
    scores = (q @ W_charge) @ charge.T  (+ const per row, softmax-invariant)
    emb    = (attn @ charge) @ W_charge.T + b_charge        (softmax sums to 1)
    pre    = (attn @ charge) @ (W_fusion @ W_charge).T + qf
      with qf = q @ W_fusion.T + b_fusion + b_charge @ W_fusion.T
    out    = sum_h' tanh(pre) * Ws + sum(bias)

Sharding: the 200 labels are split 25-per-core across 8 NeuronCores (all of
scores/attention/fusion compute is label-parallel); q-side precomputation is
tiny and done on host in fp32.
"""

import numpy as np

HID = 768
SEQ = 512
QN = 256
NL = 200
NCORES = 8
LPC = NL // NCORES  # 25 labels per core
P = 128
KH = HID // P  # 6
KS = SEQ // P  # 4
MQ = QN // P   # 2

# matmul operand dtype on device: "float32" (exact, 4 cyc/row),
# "float32r" (fast fp32, 1 cyc/row at N>=256), "bfloat16" (1 cyc/row, 2-byte)
MM_DT_NAME = "float32r"

_CACHE = {}


def _build(mm_name: str, L: int):
    import concourse.bacc as bacc
    import concourse.bass as bass
    import concourse.mybir as mybir
    from concourse.tile import TileContext

    dt = mybir.dt
    MM = getattr(dt, mm_name)
    F32 = dt.float32
    ATT = MM  # dtype of attn weights / transpose path
    Alu = mybir.AluOpType
    Act = mybir.ActivationFunctionType

    nc = bacc.Bacc("TRN2")
    d_chT = nc.dram_tensor("chT", [L, HID, SEQ], MM, kind="ExternalInput")
    d_ch = nc.dram_tensor("ch", [L, SEQ, HID], MM, kind="ExternalInput")
    d_nm = nc.dram_tensor("nm", [L, SEQ], MM, kind="ExternalInput")
    d_ws = nc.dram_tensor("ws", [L, HID], F32, kind="ExternalInput")
    d_q2T = nc.dram_tensor("q2T", [HID, QN], MM, kind="ExternalInput")
    d_web = nc.dram_tensor("wembT", [HID, HID], MM, kind="ExternalInput")
    d_qf = nc.dram_tensor("qfT", [QN, HID], F32, kind="ExternalInput")
    d_ones = nc.dram_tensor("ones", [1, P], MM, kind="ExternalInput")
    d_id = nc.dram_tensor("ident", [P, P], ATT, kind="ExternalInput")
    d_out = nc.dram_tensor("out", [MQ, P, L], F32, kind="ExternalOutput")

    with TileContext(nc) as tc:
        with (
            tc.tile_pool(name="const", bufs=1) as cpool,
            tc.tile_pool(name="io", bufs=2) as iopool,
            tc.tile_pool(name="work", bufs=2) as wpool,
            tc.tile_pool(name="ps_s", bufs=2, space="PSUM") as ps_s,
            tc.tile_pool(name="ps_t", bufs=2, space="PSUM") as ps_t,
            tc.tile_pool(name="ps_a", bufs=2, space="PSUM") as ps_a,
            tc.tile_pool(name="ps_f", bufs=1, space="PSUM") as ps_f,
        ):
            t_q2T = cpool.tile([P, KH, QN], MM)
            nc.sync.dma_start(t_q2T[:], d_q2T.rearrange("(k p) q -> p k q", p=P))
            t_web = cpool.tile([P, KH, HID], MM)
            nc.sync.dma_start(t_web[:], d_web.rearrange("(k p) h -> p k h", p=P))
            t_qf = cpool.tile([P, MQ, HID], F32)
            nc.sync.dma_start(t_qf[:], d_qf.rearrange("(m p) h -> p m h", p=P))
            t_ones = cpool.tile([1, P], MM)
            nc.sync.dma_start(t_ones[:], d_ones[:])
            t_id = cpool.tile([P, P], ATT)
            nc.sync.dma_start(t_id[:], d_id[:])

            for l in range(L):
                t_chT = iopool.tile([P, KH, SEQ], MM, tag="chT")
                nc.sync.dma_start(
                    t_chT[:], d_chT[l].rearrange("(k p) s -> p k s", p=P)
                )
                t_ch = iopool.tile([P, KS, HID], MM, tag="ch")
                nc.sync.dma_start(t_ch[:], d_ch[l].rearrange("(k p) h -> p k h", p=P))
                t_nm = iopool.tile([1, SEQ], MM, tag="nm")
                nc.gpsimd.dma_start(t_nm[:], d_nm[l : l + 1, :])
                t_ws = wpool.tile([P, HID], F32, tag="ws")
                ws_row = d_ws[l]
                ws_bcast = bass.AP(
                    tensor=ws_row.tensor,
                    offset=ws_row.offset,
                    ap=[[0, P]] + list(ws_row.ap),
                )
                nc.gpsimd.dma_start(t_ws[:], ws_bcast)

                # --- scores + masked softmax (unnormalized) ---
                t_attn = wpool.tile([P, MQ, SEQ], ATT, tag="attn")
                t_r = wpool.tile([P, MQ], F32, tag="r")
                t_recip = wpool.tile([P, MQ], F32, tag="recip")
                for m in range(MQ):
                    p_s = ps_s.tile([P, SEQ], F32, tag="ps_s")
                    for k in range(KH):
                        nc.tensor.matmul(
                            p_s[:],
                            t_q2T[:, k, m * P : (m + 1) * P],
                            t_chT[:, k, :],
                            start=(k == 0),
                            stop=False,
                        )
                    nc.tensor.matmul(
                        p_s[:], t_ones[:, :], t_nm[:, :], start=False, stop=True
                    )
                    t_nmx = wpool.tile([P, 1], F32, tag="nmx")
                    nc.vector.tensor_reduce(
                        t_nmx[:],
                        p_s[:],
                        axis=mybir.AxisListType.X,
                        op=Alu.max,
                        negate=True,
                    )
                    nc.scalar.activation(
                        t_attn[:, m, :],
                        p_s[:],
                        Act.Exp,
                        bias=t_nmx[:],
                        scale=1.0,
                        accum_out=t_r[:, m : m + 1],
                    )
                nc.vector.reciprocal(t_recip[:], t_r[:])

                # --- transpose attn -> [S, Q] tiles ---
                t_attnT = wpool.tile([P, KS, QN], MM, tag="attnT")
                for m in range(MQ):
                    for i in range(KS):
                        p_t = ps_t.tile([P, P], ATT, tag="ps_t")
                        nc.tensor.transpose(
                            p_t[:], t_attn[:, m, i * P : (i + 1) * P], t_id[:]
                        )
                        nc.vector.tensor_copy(
                            t_attnT[:, i, m * P : (m + 1) * P], p_t[:]
                        )

                # --- a_chT[h, q] = charge_l.T @ attnT ---
                t_achT = wpool.tile([P, KH, QN], MM, tag="achT")
                for mh in range(KH):
                    p_a = ps_a.tile([P, QN], F32, tag="ps_a")
                    for k in range(KS):
                        nc.tensor.matmul(
                            p_a[:],
                            t_ch[:, k, mh * P : (mh + 1) * P],
                            t_attnT[:, k, :],
                            start=(k == 0),
                            stop=(k == KS - 1),
                        )
                    nc.vector.tensor_copy(t_achT[:, mh, :], p_a[:])

                # --- fusion: pre = (a_ch/r) @ wembT + qf; out_l = sum tanh(pre)*ws ---
                t_out = wpool.tile([P, MQ], F32, tag="outcol")
                for m in range(MQ):
                    p_f = ps_f.tile([P, HID], F32, tag="ps_f")
                    for nb in range(0, HID, 512):
                        ne = min(HID, nb + 512)
                        for k in range(KH):
                            nc.tensor.matmul(
                                p_f[:, nb:ne],
                                t_achT[:, k, m * P : (m + 1) * P],
                                t_web[:, k, nb:ne],
                                start=(k == 0),
                                stop=(k == KH - 1),
                            )
                    t_fused = wpool.tile([P, HID], F32, tag="fused")
                    nc.vector.scalar_tensor_tensor(
                        t_fused[:],
                        p_f[:],
                        t_recip[:, m : m + 1],
                        t_qf[:, m, :],
                        op0=Alu.mult,
                        op1=Alu.add,
                    )
                    t_tanh = wpool.tile([P, HID], F32, tag="tanh")
                    nc.scalar.activation(t_tanh[:], t_fused[:], Act.Tanh)
                    t_scr = wpool.tile([P, HID], F32, tag="scr")
                    nc.vector.scalar_tensor_tensor(
                        t_scr[:],
                        t_tanh[:],
                        1.0,
                        t_ws[:],
                        op0=Alu.bypass,
                        op1=Alu.mult,
                        accum_out=t_out[:, m : m + 1],
                    )
                nc.sync.dma_start(
                    d_out.rearrange("t p l -> p t l")[:, :, l], t_out[:]
                )

    nc.compile()
    return nc


def _get_nc(mm_name: str, L: int):
    key = (mm_name, L)
    if key not in _CACHE:
        _CACHE[key] = _build(mm_name, L)
    return _CACHE[key]


def _host_prep(Q_fact, charge, charge_mask, W_fact, b_fact, W_charge, b_charge,
               W_fusion, b_fusion, Ws, bias, mm_name):
    f32 = np.float32
    q = (Q_fact.astype(f32) @ W_fact.T.astype(f32)) + b_fact.astype(f32)
    q2T = np.ascontiguousarray((q @ W_charge.astype(f32)).T)
    qf = (
        q @ W_fusion.T.astype(f32)
        + b_fusion.astype(f32)
        + (b_charge.astype(f32) @ W_fusion.T.astype(f32))
    )
    wembT = np.ascontiguousarray(
        (W_fusion.astype(np.float64) @ W_charge.astype(np.float64)).T
    ).astype(f32)
    negm = ((1.0 - charge_mask.astype(f32)) * f32(-1e9)).astype(f32)
    chT = np.ascontiguousarray(charge.transpose(0, 2, 1)).astype(f32)
    bias_sum = f32(bias.astype(np.float64).sum())

    if mm_name == "bfloat16":
        import ml_dtypes

        cast = lambda x: np.ascontiguousarray(x).astype(ml_dtypes.bfloat16)
    else:
        cast = lambda x: np.ascontiguousarray(x, dtype=f32)

    shared = {
        "q2T": cast(q2T),
        "wembT": cast(wembT),
        "qfT": np.ascontiguousarray(qf, dtype=f32),
        "ones": cast(np.ones((1, P), dtype=f32)),
        "ident": cast(np.eye(P, dtype=f32)),
    }
    per_core = []
    for c in range(NCORES):
        sl = slice(c * LPC, (c + 1) * LPC)
        m = dict(shared)
        m["chT"] = cast(chT[sl])
        m["ch"] = cast(charge[sl].astype(f32))
        m["nm"] = cast(negm[sl])
        m["ws"] = np.ascontiguousarray(Ws[sl], dtype=f32)
        per_core.append(m)
    return per_core, bias_sum


def kernel(Q_fact, charge, charge_mask, W_fact, b_fact, W_charge, b_charge,
           W_fusion, b_fusion, Ws, bias):
    from concourse.bass_utils import run_bass_kernel_spmd

    mm_name = MM_DT_NAME
    nc = _get_nc(mm_name, LPC)
    in_maps, bias_sum = _host_prep(
        Q_fact, charge, charge_mask, W_fact, b_fact, W_charge, b_charge,
        W_fusion, b_fusion, Ws, bias, mm_name,
    )
    res = run_bass_kernel_spmd(nc, in_maps, list(range(NCORES)))
    cols = [res.results[i]["out"].reshape(QN, LPC) for i in range(NCORES)]
    out = np.concatenate(cols, axis=1) + bias_sum
    return np.ascontiguousarray(out, dtype=np.float32)


# revision 17
# speedup vs baseline: 128.5487x; 128.5487x over previous
"""Trainium2 Bass kernel for nn_Charge_Fusion (cross-attention charge fusion).

Math (reference, per fact q and label c):
    q    = Q_fact @ W_fact.T + b_fact                       [Q, H]
    cemb = charge @ W_charge.T + b_charge                   [C, S, H]
    attn = softmax_s(q . cemb + mask)                       [Q, C, S]
    emb  = attn @ cemb                                      [Q, C, H]
    out  = sum_h(tanh((q + emb) @ W_fusion.T + b_fusion) * Ws + bias)   [Q, C]

Algebraic rewrite used here (saves the dominant 121-GF charge projection):
    scores = (q @ W_charge) @ charge.T  (+ const per row, softmax-invariant)
    emb    = (attn @ charge) @ W_charge.T + b_charge        (softmax sums to 1)
    pre    = (attn @ charge) @ (W_fusion @ W_charge).T + qf
      with qf = q @ W_fusion.T + b_fusion + b_charge @ W_fusion.T
    out    = sum_h' tanh(pre) * Ws + sum(bias)

Sharding: the 200 labels are split 25-per-core across 8 NeuronCores (all of
scores/attention/fusion compute is label-parallel); q-side precomputation is
tiny and done on host in fp32.
"""

import numpy as np

HID = 768
SEQ = 512
QN = 256
NL = 200
NCORES = 8
LPC = NL // NCORES  # 25 labels per core
P = 128
KH = HID // P  # 6
KS = SEQ // P  # 4
MQ = QN // P   # 2

# matmul operand dtype on device: "float32" (exact, 4 cyc/row),
# "float32r" (fast fp32, 1 cyc/row at N>=256), "bfloat16" (1 cyc/row, 2-byte)
MM_DT_NAME = "float32r"

_CACHE = {}

# schedule-tuning knobs (io_bufs, work_bufs, ps_s, ps_t, ps_a, ps_f, acht_on_act)
CFG = dict(io=2, work=2, ps_s=2, ps_t=2, ps_a=2, ps_f=1, acht_act=False,
           ch_bf16=False, ws_bcast_dma=True)


def _build(mm_name: str, L: int):
    import concourse.bacc as bacc
    import concourse.bass as bass
    import concourse.mybir as mybir
    from concourse.tile import TileContext

    dt = mybir.dt
    MM = getattr(dt, mm_name)
    F32 = dt.float32
    # attn / ch path dtype: bf16 halves the `ch` DMA stream and speeds the
    # transpose; the averaging path tolerates it
    ATT = dt.bfloat16 if CFG["ch_bf16"] else MM
    Alu = mybir.AluOpType
    Act = mybir.ActivationFunctionType

    nc = bacc.Bacc("TRN2")
    d_chT = nc.dram_tensor("chT", [L, HID, SEQ], MM, kind="ExternalInput")
    d_ch = nc.dram_tensor("ch", [L, SEQ, HID], ATT, kind="ExternalInput")
    d_nm = nc.dram_tensor("nm", [L, SEQ], MM, kind="ExternalInput")
    d_ws = nc.dram_tensor("ws", [L, HID], F32, kind="ExternalInput")
    d_q2T = nc.dram_tensor("q2T", [HID, QN], MM, kind="ExternalInput")
    d_web = nc.dram_tensor("wembT", [HID, HID], MM, kind="ExternalInput")
    d_qf = nc.dram_tensor("qfT", [QN, HID], F32, kind="ExternalInput")
    d_ones = nc.dram_tensor("ones", [1, P], MM, kind="ExternalInput")
    d_id = nc.dram_tensor("ident", [P, P], ATT, kind="ExternalInput")
    d_out = nc.dram_tensor("out", [MQ, P, L], F32, kind="ExternalOutput")

    with TileContext(nc) as tc:
        with (
            tc.tile_pool(name="const", bufs=1) as cpool,
            tc.tile_pool(name="io", bufs=CFG["io"]) as iopool,
            tc.tile_pool(name="work", bufs=CFG["work"]) as wpool,
            tc.tile_pool(name="ps_s", bufs=CFG["ps_s"], space="PSUM") as ps_s,
            tc.tile_pool(name="ps_t", bufs=CFG["ps_t"], space="PSUM") as ps_t,
            tc.tile_pool(name="ps_a", bufs=CFG["ps_a"], space="PSUM") as ps_a,
            tc.tile_pool(name="ps_f", bufs=CFG["ps_f"], space="PSUM") as ps_f,
        ):
            def load_label(l, chunk_chT=False):
                t_chT = iopool.tile([P, KH, SEQ], MM, tag="chT")
                chT_src = d_chT[l].rearrange("(k p) s -> p k s", p=P)
                if chunk_chT:
                    for k in range(KH):
                        nc.sync.dma_start(t_chT[:, k, :], chT_src[:, k, :])
                else:
                    nc.sync.dma_start(t_chT[:], chT_src)
                t_ch = iopool.tile([P, KS, HID], ATT, tag="ch")
                nc.sync.dma_start(t_ch[:], d_ch[l].rearrange("(k p) h -> p k h", p=P))
                t_nm = iopool.tile([1, SEQ], MM, tag="nm")
                nc.gpsimd.dma_start(t_nm[:], d_nm[l : l + 1, :])
                t_ws = wpool.tile([P, HID], F32, tag="ws")
                if CFG["ws_bcast_dma"]:
                    ws_row = d_ws[l]
                    ws_bcast = bass.AP(
                        tensor=ws_row.tensor,
                        offset=ws_row.offset,
                        ap=[[0, P]] + list(ws_row.ap),
                    )
                    nc.gpsimd.dma_start(t_ws[:], ws_bcast)
                else:
                    nc.gpsimd.partition_broadcast(
                        t_ws[:], t_wsall[0:1, l * HID : (l + 1) * HID]
                    )
                return t_chT, t_ch, t_nm, t_ws

            # startup order: only what scores(0) needs first, big fusion-time
            # constants after label 0's inputs
            t_q2T = cpool.tile([P, KH, QN], MM)
            nc.sync.dma_start(t_q2T[:], d_q2T.rearrange("(k p) q -> p k q", p=P))
            t_ones = cpool.tile([1, P], MM)
            nc.sync.dma_start(t_ones[:], d_ones[:])
            if not CFG["ws_bcast_dma"]:
                t_wsall = cpool.tile([1, L * HID], F32)
                nc.sync.dma_start(t_wsall[:], d_ws.rearrange("l h -> (l h)")[None, :])
            pre_loaded = load_label(0, chunk_chT=True)
            t_id = cpool.tile([P, P], ATT)
            nc.sync.dma_start(t_id[:], d_id[:])
            t_web = cpool.tile([P, KH, HID], MM)
            nc.sync.dma_start(t_web[:], d_web.rearrange("(k p) h -> p k h", p=P))
            t_qf = cpool.tile([P, MQ, HID], F32)
            nc.sync.dma_start(t_qf[:], d_qf.rearrange("(m p) h -> p m h", p=P))

            for l in range(L):
                t_chT, t_ch, t_nm, t_ws = (
                    pre_loaded if l == 0 else load_label(l)
                )

                # --- scores + masked softmax (unnormalized) ---
                t_attn = wpool.tile([P, MQ, SEQ], ATT, tag="attn")
                t_r = wpool.tile([P, MQ], F32, tag="r")
                t_recip = wpool.tile([P, MQ], F32, tag="recip")
                for m in range(MQ):
                    p_s = ps_s.tile([P, SEQ], F32, tag="ps_s")
                    for k in range(KH):
                        nc.tensor.matmul(
                            p_s[:],
                            t_q2T[:, k, m * P : (m + 1) * P],
                            t_chT[:, k, :],
                            start=(k == 0),
                            stop=False,
                        )
                    nc.tensor.matmul(
                        p_s[:], t_ones[:, :], t_nm[:, :], start=False, stop=True
                    )
                    t_nmx = wpool.tile([P, 1], F32, tag="nmx")
                    nc.vector.tensor_reduce(
                        t_nmx[:],
                        p_s[:],
                        axis=mybir.AxisListType.X,
                        op=Alu.max,
                        negate=True,
                    )
                    nc.scalar.activation(
                        t_attn[:, m, :],
                        p_s[:],
                        Act.Exp,
                        bias=t_nmx[:],
                        scale=1.0,
                        accum_out=t_r[:, m : m + 1],
                    )
                nc.vector.reciprocal(t_recip[:], t_r[:])

                # --- transpose attn -> [S, Q] tiles (4 transposes per PSUM bank,
                # one batched eviction copy per m) ---
                t_attnT = wpool.tile([P, KS, QN], ATT, tag="attnT")
                for m in range(MQ):
                    p_t = ps_t.tile([P, SEQ], ATT, tag="ps_t")
                    for i in range(KS):
                        nc.tensor.transpose(
                            p_t[:, i * P : (i + 1) * P],
                            t_attn[:, m, i * P : (i + 1) * P],
                            t_id[:],
                        )
                    nc.vector.tensor_copy(
                        t_attnT[:, :, m * P : (m + 1) * P],
                        p_t[:].rearrange("p (i q) -> p i q", i=KS),
                    )

                # --- a_chT[h, q] = charge_l.T @ attnT (mh pairs share a PSUM
                # bank; batched eviction on the scalar engine) ---
                t_achT = wpool.tile([P, KH, QN], MM, tag="achT")
                for j in range(KH // 2):
                    p_a = ps_a.tile([P, 2 * QN], F32, tag="ps_a")
                    for h in range(2):
                        mh = 2 * j + h
                        for k in range(KS):
                            nc.tensor.matmul(
                                p_a[:, h * QN : (h + 1) * QN],
                                t_ch[:, k, mh * P : (mh + 1) * P],
                                t_attnT[:, k, :],
                                start=(k == 0),
                                stop=(k == KS - 1),
                            )
                    _cp = nc.scalar.copy if CFG["acht_act"] else nc.vector.tensor_copy
                    _cp(
                        t_achT[:, 2 * j : 2 * j + 2, :],
                        p_a[:].rearrange("p (j q) -> p j q", j=2),
                    )

                # --- fusion: pre = (a_ch/r) @ wembT + qf; out_l = sum tanh(pre)*ws ---
                t_out = wpool.tile([P, MQ], F32, tag="outcol")
                for m in range(MQ):
                    p_f = ps_f.tile([P, HID], F32, tag="ps_f")
                    for nb in range(0, HID, 512):
                        ne = min(HID, nb + 512)
                        for k in range(KH):
                            nc.tensor.matmul(
                                p_f[:, nb:ne],
                                t_achT[:, k, m * P : (m + 1) * P],
                                t_web[:, k, nb:ne],
                                start=(k == 0),
                                stop=(k == KH - 1),
                            )
                    t_fused = wpool.tile([P, HID], F32, tag="fused")
                    nc.vector.scalar_tensor_tensor(
                        t_fused[:],
                        p_f[:],
                        t_recip[:, m : m + 1],
                        t_qf[:, m, :],
                        op0=Alu.mult,
                        op1=Alu.add,
                    )
                    t_tanh = wpool.tile([P, HID], F32, tag="tanh")
                    nc.scalar.activation(t_tanh[:], t_fused[:], Act.Tanh)
                    t_scr = wpool.tile([P, HID], F32, tag="scr")
                    nc.vector.scalar_tensor_tensor(
                        t_scr[:],
                        t_tanh[:],
                        1.0,
                        t_ws[:],
                        op0=Alu.bypass,
                        op1=Alu.mult,
                        accum_out=t_out[:, m : m + 1],
                    )
                nc.sync.dma_start(
                    d_out.rearrange("t p l -> p t l")[:, :, l], t_out[:]
                )

    nc.compile()
    return nc


def _get_nc(mm_name: str, L: int):
    key = (mm_name, L, tuple(sorted(CFG.items())))
    if key not in _CACHE:
        _CACHE[key] = _build(mm_name, L)
    return _CACHE[key]


def _host_prep(Q_fact, charge, charge_mask, W_fact, b_fact, W_charge, b_charge,
               W_fusion, b_fusion, Ws, bias, mm_name):
    import ml_dtypes
    att_cast = (
        (lambda x: np.ascontiguousarray(x).astype(ml_dtypes.bfloat16))
        if CFG["ch_bf16"]
        else (lambda x: np.ascontiguousarray(x, dtype=np.float32))
    )
    f32 = np.float32
    q = (Q_fact.astype(f32) @ W_fact.T.astype(f32)) + b_fact.astype(f32)
    q2T = np.ascontiguousarray((q @ W_charge.astype(f32)).T)
    qf = (
        q @ W_fusion.T.astype(f32)
        + b_fusion.astype(f32)
        + (b_charge.astype(f32) @ W_fusion.T.astype(f32))
    )
    wembT = np.ascontiguousarray(
        (W_fusion.astype(np.float64) @ W_charge.astype(np.float64)).T
    ).astype(f32)
    negm = ((1.0 - charge_mask.astype(f32)) * f32(-1e9)).astype(f32)
    chT = np.ascontiguousarray(charge.transpose(0, 2, 1)).astype(f32)
    bias_sum = f32(bias.astype(np.float64).sum())

    if mm_name == "bfloat16":
        cast = lambda x: np.ascontiguousarray(x).astype(ml_dtypes.bfloat16)
        att_cast = cast
    else:
        cast = lambda x: np.ascontiguousarray(x, dtype=f32)

    shared = {
        "q2T": cast(q2T),
        "wembT": cast(wembT),
        "qfT": np.ascontiguousarray(qf, dtype=f32),
        "ones": cast(np.ones((1, P), dtype=f32)),
        "ident": att_cast(np.eye(P, dtype=f32)),
    }
    per_core = []
    for c in range(NCORES):
        sl = slice(c * LPC, (c + 1) * LPC)
        m = dict(shared)
        m["chT"] = cast(chT[sl])
        m["ch"] = att_cast(charge[sl].astype(f32))
        m["nm"] = cast(negm[sl])
        m["ws"] = np.ascontiguousarray(Ws[sl], dtype=f32)
        per_core.append(m)
    return per_core, bias_sum


def kernel(Q_fact, charge, charge_mask, W_fact, b_fact, W_charge, b_charge,
           W_fusion, b_fusion, Ws, bias):
    from concourse.bass_utils import run_bass_kernel_spmd

    mm_name = MM_DT_NAME
    nc = _get_nc(mm_name, LPC)
    in_maps, bias_sum = _host_prep(
        Q_fact, charge, charge_mask, W_fact, b_fact, W_charge, b_charge,
        W_fusion, b_fusion, Ws, bias, mm_name,
    )
    res = run_bass_kernel_spmd(nc, in_maps, list(range(NCORES)))
    cols = [res.results[i]["out"].reshape(QN, LPC) for i in range(NCORES)]
    out = np.concatenate(cols, axis=1) + bias_sum
    return np.ascontiguousarray(out, dtype=np.float32)
